# revision 27
# baseline (speedup 1.0000x reference)
"""Tensor-parallel DeepSpeed-style self-attention block on 8 TRN2 NeuronCores.

Strategy (head-sharded QKV/attention + all-to-all + token-sharded output GEMM):
  - LN gamma/beta folded into QKV weight/bias on host; x, W, OW shipped bf16.
  - Each core owns 2 of 16 heads: computes z=(x-mu)*istd (replicated), z^T via
    PE transposes (batched evictions), then Q^T,K^T (transposed layout, bias
    added during ACT eviction) and V (natural layout, bias via K=1 matmul).
  - Attention per (batch, head) in TRANSPOSED score layout: sT[k,q] = K·Q^T
    computed directly per 128-k-block x 512-q-superblock; exp on ACT writes
    pT[k,q] straight to SBUF (no transposes); rowsums accumulate via a
    ones-column matmul; ctx^T[d,q] accumulates via lhsT=V chunks with 512-wide
    moving dim; normalization = reciprocal + ones-row outer-product broadcast
    + one DVE multiply per (b,h,q-superblock).
  - AllToAll (one per batch, bf16, 1MB, Shared-HBM output) redistributes ctx^T
    from head-sharded to token-sharded; A2A(b0) overlaps attention(b1) and
    A2A(b1) overlaps the b0 output GEMM.
  - Output GEMM: full attn_ow (replicated bf16) x token shard. Each core
    writes a [512, 2048] f32 output shard; host concatenates.
"""

import sys

if "/opt/trn_rl_repo" not in sys.path:
    sys.path.insert(0, "/opt/trn_rl_repo")

# --- shim antenv.axon_hooks (missing in this image) so trace=True can NTFF-profile ---
import types, ctypes, contextlib


def _make_ntff_hook(so_path="/opt/axon/libaxon_pjrt.so"):
    try:
        lib = ctypes.CDLL(so_path)
    except OSError:
        return None
    if not hasattr(lib, "axon_start_nrt_profile"):
        return None
    lib.axon_start_nrt_profile.argtypes = [ctypes.POINTER(ctypes.c_int64), ctypes.c_size_t]
    lib.axon_start_nrt_profile.restype = ctypes.c_int64
    lib.axon_stop_nrt_profile.argtypes = [ctypes.c_char_p]
    lib.axon_stop_nrt_profile.restype = ctypes.c_int64

    @contextlib.contextmanager
    def _hook(output_dir, device_ids):
        import jax

        jax.devices()
        if device_ids:
            ids = (ctypes.c_int64 * len(device_ids))(*device_ids)
            rc = lib.axon_start_nrt_profile(ids, len(device_ids))
        else:
            rc = lib.axon_start_nrt_profile(None, 0)
        if rc != 0:
            raise RuntimeError(f"axon_start_nrt_profile rc={rc}")
        try:
            yield
        finally:
            n = lib.axon_stop_nrt_profile(str(output_dir).encode())
            if n < 0:
                raise RuntimeError(f"axon_stop_nrt_profile rc={n}")

    return _hook


if "antenv.axon_hooks" not in sys.modules:
    _m = types.ModuleType("antenv.axon_hooks")
    _m.get_axon_ntff_profile_hook = lambda: _make_ntff_hook()
    sys.modules["antenv.axon_hooks"] = _m
# --- end shim ---

import numpy as np
import ml_dtypes  # noqa: F401  (bf16 numpy dtype registration)

from concourse import bacc, tile, mybir
from concourse.masks import make_identity

B, S, HID = 2, 2048, 2048
HEADS = 16
HD = 128  # head dim
T = B * S  # 4096 tokens
N_CORES = 8
HPC = HEADS // N_CORES  # 2 heads per core
EPS = 1e-6
SCALE = 1.0 / float(np.sqrt(HD))
NEG = -1e9

F32 = mybir.dt.float32
BF16 = mybir.dt.bfloat16

TOKB = 128  # token block (partition dim)
SB = 512  # superblock of tokens for QKV GEMM / attention q dim
N_SB = T // SB  # 8
N_TB = SB // TOKB  # 4
N_CC = HID // 128  # 16 contraction chunks
N_KB = S // TOKB  # 16 key blocks per batch
TOK_SHARD = S // N_CORES  # 256 tokens per (batch, core) after A2A

Act = mybir.ActivationFunctionType
Alu = mybir.AluOpType


def _build(apply_mask: bool):
    nc = bacc.Bacc("TRN2", target_bir_lowering=False, debug=False, num_devices=N_CORES)

    inp = nc.dram_tensor("input", [T, HID], BF16, kind="ExternalInput").ap()
    wqkv = nc.dram_tensor("qkvw", [HID, 3 * HPC * HD], BF16, kind="ExternalInput").ap()
    qkb = nc.dram_tensor("qkb", [128, 2 * HPC], F32, kind="ExternalInput").ap()
    vb = nc.dram_tensor("vb", [1, HPC * HD], BF16, kind="ExternalInput").ap()
    ow = nc.dram_tensor("ow", [HID, HID], BF16, kind="ExternalInput").ap()
    out = nc.dram_tensor("out", [B * TOK_SHARD, HID], F32, kind="ExternalOutput").ap()
    if apply_mask:
        # [p, b*N_KB + kb] = mask[b, kb*128 + p]  (per-key additive mask)
        imask = nc.dram_tensor("imask", [128, B * N_KB], F32, kind="ExternalInput").ap()

    # one contiguous A2A buffer per (batch, head) so each head's collective can
    # start as soon as that head's attention finishes
    cc_in = [
        [nc.dram_tensor(f"cc_in{b}_{h}", [N_CORES, HD, TOK_SHARD], BF16).ap() for h in range(HPC)]
        for b in range(B)
    ]
    cc_out = [
        [nc.dram_tensor(f"cc_out{b}_{h}", [N_CORES, HD, TOK_SHARD], BF16).ap() for h in range(HPC)]
        for b in range(B)
    ]

    with tile.TileContext(nc) as tc:
        with tc.tile_pool(name="persist", bufs=1) as pers:
            ident = pers.tile([128, 128], BF16)
            make_identity(nc, ident[:])
            # causalT[k, q] = 0 where k <= q else NEG (transposed causal mask)
            causalT = pers.tile([128, 128], F32)
            nc.gpsimd.memset(causalT[:], 0.0)
            nc.gpsimd.affine_select(
                out=causalT[:],
                in_=causalT[:],
                compare_op=Alu.is_ge,
                fill=NEG,
                base=0,
                # keep where (-k + q) >= 0, i.e. q >= k
                pattern=[[1, 128]],
                channel_multiplier=-1,
            )
            ones_col = pers.tile([128, 1], BF16)
            nc.gpsimd.memset(ones_col[:], 1.0)
            ones_row = pers.tile([1, 128], BF16)
            nc.gpsimd.memset(ones_row[:], 1.0)
            eps_t = pers.tile([128, 1], F32)
            nc.gpsimd.memset(eps_t[:], EPS)
            qkb_sb = pers.tile([128, 2 * HPC], F32)
            nc.sync.dma_start(out=qkb_sb[:], in_=qkb[:])
            vb_sb = pers.tile([1, HPC * HD], BF16)
            nc.sync.dma_start(out=vb_sb[:], in_=vb[:])

            qT = pers.tile([128, HPC, T], BF16)  # [d, head, tok]
            kT = pers.tile([128, HPC, T], BF16)
            v_sb = pers.tile([128, T // 128, HPC * HD], BF16)  # [tok128, blk, hcol]

            if apply_mask:
                msk = pers.tile([128, B * N_KB], F32)
                nc.sync.dma_start(out=msk[:], in_=imask[:])

            # ---------------- Phase A: LN + z^T + QKV GEMM ----------------
            with (
                tc.tile_pool(name="pa_w", bufs=1) as paw,
                tc.tile_pool(name="pa_x", bufs=6) as px,
                tc.tile_pool(name="pa_st", bufs=4) as pst,
                tc.tile_pool(name="pa_z", bufs=3) as pz,
                tc.tile_pool(name="pa_zT", bufs=2) as pzT,
                tc.tile_pool(name="pa_tr", bufs=3, space="PSUM") as ptr,
                tc.tile_pool(name="pa_qk", bufs=2, space="PSUM") as pqk,
                tc.tile_pool(name="pa_v", bufs=2, space="PSUM") as ppv,
            ):
                # prefetch x for the first superblock BEFORE the weight DMAs so
                # compute can start immediately; the first tile is split in four
                # so its bn_stats (and the first transposes) start ASAP
                pre_x = []
                for tb in range(N_TB):
                    x_t = px.tile([128, HID], BF16, tag="x")
                    if tb == 0:
                        for c4 in range(4):
                            nc.sync.dma_start(
                                out=x_t[:, c4 * 512 : (c4 + 1) * 512],
                                in_=inp[tb * TOKB : tb * TOKB + 128, c4 * 512 : (c4 + 1) * 512],
                            )
                    else:
                        nc.sync.dma_start(out=x_t[:], in_=inp[tb * TOKB : tb * TOKB + 128, :])
                    pre_x.append(x_t)

                w_sb = paw.tile([128, N_CC, 3 * HPC * HD], BF16)
                for cc in range(N_CC):
                    nc.sync.dma_start(out=w_sb[:, cc, :], in_=wqkv[cc * 128 : (cc + 1) * 128, :])

                for sb in range(N_SB):
                    zT = pzT.tile([128, N_CC, SB], BF16)
                    for tb in range(N_TB):
                        r0 = sb * SB + tb * TOKB
                        if sb == 0:
                            x_t = pre_x[tb]
                        else:
                            x_t = px.tile([128, HID], BF16, tag="x")
                            nc.sync.dma_start(out=x_t[:], in_=inp[r0 : r0 + 128, :])
                        bn = pst.tile([128, 4, 6], F32, tag="bn")
                        for c4 in range(4):
                            nc.vector.bn_stats(bn[:, c4, :], x_t[:, c4 * 512 : (c4 + 1) * 512])
                        mv = pst.tile([128, 2], F32, tag="mv")
                        nc.vector.bn_aggr(mv[:], bn[:])
                        sd = pst.tile([128, 1], F32, tag="sd")
                        nc.scalar.activation(sd[:], mv[:, 1:2], Act.Sqrt, bias=eps_t[:])
                        istd = pst.tile([128, 1], F32, tag="istd")
                        nc.vector.reciprocal(istd[:], sd[:])
                        z_t = pz.tile([128, HID], BF16)
                        nc.vector.tensor_scalar(
                            out=z_t[:],
                            in0=x_t[:],
                            scalar1=mv[:, 0:1],
                            scalar2=istd[:],
                            op0=Alu.subtract,
                            op1=Alu.mult,
                        )
                        # transpose 4 cc-chunks into one PSUM tile, single eviction
                        for ccg in range(4):
                            zt_ps = ptr.tile([128, 4, 128], BF16)
                            for i in range(4):
                                cc = ccg * 4 + i
                                nc.tensor.transpose(
                                    zt_ps[:, i, :], z_t[:, cc * 128 : (cc + 1) * 128], ident[:]
                                )
                            dst = zT[:, ccg * 4 : ccg * 4 + 4, tb * TOKB : tb * TOKB + 128]
                            if ccg % 2 == 0:
                                nc.scalar.copy(dst, zt_ps[:])
                            else:
                                nc.vector.tensor_copy(dst, zt_ps[:])

                    # Q^T, K^T for this superblock (transposed GEMM); bias folded
                    # into the ACT eviction (Identity with per-partition bias)
                    for h in range(HPC):
                        for base, bias_col, dst in (
                            (0, h, qT),
                            (HPC * HD, HPC + h, kT),
                        ):
                            psq = pqk.tile([128, SB], F32)
                            for cc in range(N_CC):
                                nc.tensor.matmul(
                                    psq[:],
                                    w_sb[:, cc, base + h * HD : base + (h + 1) * HD],
                                    zT[:, cc, :],
                                    start=(cc == 0),
                                    stop=(cc == N_CC - 1),
                                )
                            nc.scalar.activation(
                                dst[:, h, sb * SB : (sb + 1) * SB],
                                psq[:],
                                Act.Identity,
                                bias=qkb_sb[:, bias_col : bias_col + 1],
                            )
                    # V natural (bias via K=1 ones-row matmul opening the group)
                    for tb in range(N_TB):
                        psv = ppv.tile([128, HPC * HD], F32)
                        nc.tensor.matmul(psv[:], ones_row[:], vb_sb[:], start=True, stop=False)
                        for cc in range(N_CC):
                            nc.tensor.matmul(
                                psv[:],
                                zT[:, cc, tb * TOKB : tb * TOKB + 128],
                                w_sb[:, cc, 2 * HPC * HD :],
                                start=False,
                                stop=(cc == N_CC - 1),
                            )
                        if tb % 2 == 0:
                            nc.scalar.copy(v_sb[:, sb * N_TB + tb, :], psv[:])
                        else:
                            nc.vector.tensor_copy(v_sb[:, sb * N_TB + tb, :], psv[:])

            # ------------- Phase B/C: attention, A2A, output GEMM -------------
            with (
                tc.tile_pool(name="pb_ow", bufs=1) as pow_,
                tc.tile_pool(name="pb_p", bufs=6) as pp,
                tc.tile_pool(name="pb_st", bufs=2) as pbs,
                tc.tile_pool(name="pb_ctxT", bufs=2) as pcT,
                tc.tile_pool(name="pb_cf", bufs=2) as pcf,
                tc.tile_pool(name="pb_o", bufs=4) as po,
                tc.tile_pool(name="ps_sc", bufs=3, space="PSUM") as pssc,
                tc.tile_pool(name="ps_ctx", bufs=2, space="PSUM") as psctx,
                tc.tile_pool(name="ps_rs", bufs=2, space="PSUM") as psrs,
                tc.tile_pool(name="ps_o", bufs=1, space="PSUM") as psout,
            ):
                # OW is bf16 in DRAM: straight DMA, no casts; overlaps attention
                ow_sb = pow_.tile([128, N_CC, HID], BF16)
                for cc in range(N_CC):
                    nc.sync.dma_start(out=ow_sb[:, cc, :], in_=ow[cc * 128 : (cc + 1) * 128, :])

                cf_tiles = [None, None]

                def emit_sc(b, h, qs, kb):
                    """scores + exp for one k-block; returns the pT tile + qlo."""
                    j = kb - 4 * qs
                    qlo = j * TOKB if j > 0 else 0
                    q0 = b * S + qs * SB
                    ps = pssc.tile([128, SB], F32, tag="sc")
                    nc.tensor.matmul(
                        ps[:, qlo:],
                        kT[:, h, b * S + kb * TOKB : b * S + kb * TOKB + 128],
                        qT[:, h, q0 + qlo : q0 + SB],
                        start=True,
                        stop=True,
                    )
                    if j >= 0:
                        nc.vector.tensor_add(
                            ps[:, j * TOKB : (j + 1) * TOKB],
                            ps[:, j * TOKB : (j + 1) * TOKB],
                            causalT[:],
                        )
                    p_t = pp.tile([128, SB], BF16, tag="p")
                    bias = msk[:, b * N_KB + kb : b * N_KB + kb + 1] if apply_mask else 0.0
                    nc.scalar.activation(p_t[:, qlo:], ps[:, qlo:], Act.Exp, scale=SCALE, bias=bias)
                    return p_t, qlo

                class Stream:
                    """One (qs) causal-attention group, software-pipelined."""

                    def __init__(self, b, h, qs):
                        self.b, self.h, self.qs = b, h, qs
                        self.nkb = 4 * (qs + 1)
                        self.kb = 0
                        self.rs = psrs.tile([1, SB], F32, tag="rs")
                        self.psc = psctx.tile([128, SB], F32, tag="ctx")
                        self.prev = emit_sc(b, h, qs, 0)

                    def step(self):
                        """Emit one k-block's consumers (+ next block's scores)."""
                        kb = self.kb
                        cur = self.prev
                        if kb + 1 < self.nkb:
                            self.prev = emit_sc(self.b, self.h, self.qs, kb + 1)
                        p_t, qlo = cur
                        flags = dict(start=(kb == 0), stop=(kb == self.nkb - 1))
                        nc.tensor.matmul(self.rs[0:1, qlo:], ones_col[:], p_t[:, qlo:], **flags)
                        nc.tensor.matmul(
                            self.psc[:, qlo:],
                            v_sb[:, self.b * N_KB + kb, self.h * HD : (self.h + 1) * HD],
                            p_t[:, qlo:],
                            **flags,
                        )
                        self.kb += 1
                        if self.kb < self.nkb:
                            return True
                        # stream done: reciprocal of the rowsums right away (DVE),
                        # so the deferred broadcast matmul never waits on it
                        recip_f = pbs.tile([1, SB], F32, tag="rf")
                        nc.vector.reciprocal_approx_fast(recip_f[:], self.rs[:])
                        self.recip_b = pbs.tile([1, SB], BF16, tag="rb")
                        nc.vector.tensor_copy(self.recip_b[:], recip_f[:])
                        return False

                    def finalize(self, ctxT):
                        """Deferred normalization: emitted after the next pair's
                        opening scores so the PE pipeline never drains."""
                        bc = pssc.tile([128, SB], F32, tag="sc")
                        nc.tensor.matmul(bc[:], ones_row[:], self.recip_b[:], start=True, stop=True)
                        bc_sb = pbs.tile([128, SB], BF16, tag="bc")
                        nc.vector.tensor_copy(bc_sb[:], bc[:], )
                        nc.vector.tensor_mul(
                            ctxT[:, self.h, self.qs * SB : (self.qs + 1) * SB],
                            self.psc[:],
                            bc_sb[:],
                        )

                def ship_head(b, h, ctxT):
                    """DMA one head's ctx^T out and kick its AllToAll chunk."""
                    for j in range(N_CORES):
                        nc.sync.dma_start(
                            out=cc_in[b][h][j, :, :],
                            in_=ctxT[:, h, j * TOK_SHARD : (j + 1) * TOK_SHARD],
                        )
                    nc.gpsimd.collective_compute(
                        "AllToAll",
                        Alu.bypass,
                        replica_groups=[list(range(N_CORES))],
                        ins=[cc_in[b][h][:]],
                        outs=[cc_out[b][h][:]],
                    )

                def emit_out_group(b, tb, nb, cf):
                    """One output-GEMM PSUM group: pure PE work, no ACT deps."""
                    pso_t = psout.tile([128, SB], F32)
                    for cc in range(N_CC):
                        nc.tensor.matmul(
                            pso_t[:],
                            cf[:, cc, tb * TOKB : tb * TOKB + 128],
                            ow_sb[:, cc, nb * 512 : (nb + 1) * 512],
                            start=(cc == 0),
                            stop=(cc == N_CC - 1),
                        )
                    o_t = po.tile([128, 512], F32)
                    if nb % 2 == 0:
                        nc.scalar.copy(o_t[:], pso_t[:])
                    else:
                        nc.vector.tensor_copy(o_t[:], pso_t[:])
                    nc.sync.dma_start(
                        out=out[b * TOK_SHARD + tb * TOKB : b * TOK_SHARD + tb * TOKB + 128,
                                nb * 512 : (nb + 1) * 512],
                        in_=o_t[:],
                    )

                out_filler = []  # deferred b=0 out-GEMM groups, drained late in attn b=1
                for b in range(B):
                    ctxT = pcT.tile([128, HPC, S], BF16)
                    # two q-superblock streams interleaved per pair: while one
                    # stream's exp runs on ACT the PE works the other stream.
                    # Pairs are pipelined into each other: the next pair's first
                    # k-blocks issue before the previous pair's normalization, so
                    # the PE never drains (and stays at its ramped pstate).
                    pending = None  # (streams, head_done)
                    pair_idx = 0
                    for h in range(HPC):
                        for pi, pair in enumerate(((0, 1), (2, 3))):
                            streams = [Stream(b, h, qs) for qs in pair]
                            active = [st for st in streams if st.step()]
                            if pending is not None:
                                pstreams, phead = pending
                                for st in pstreams:
                                    st.finalize(ctxT)
                                if phead is not None:
                                    ship_head(b, phead, ctxT)
                            while active:
                                active = [st for st in active if st.step()]
                            pending = (streams, h if pi == 1 else None)
                            # feed b=0 output-GEMM groups into the tail of b=1's
                            # attention as ACT-free PE filler; start only once the
                            # b0 collective has certainly landed (pair 2+)
                            if b == 1 and pair_idx >= 2:
                                for _ in range(2):
                                    if out_filler:
                                        out_filler.pop(0)()
                            pair_idx += 1
                    pstreams, phead = pending
                    for st in pstreams:
                        st.finalize(ctxT)
                    ship_head(b, phead, ctxT)
                    # prefetch this batch's gathered ctx^T right away so the
                    # b=0 loads overlap attention on b=1
                    cf = pcf.tile([128, N_CC, TOK_SHARD], BF16, tag="cf")
                    for cc in range(N_CC):
                        nc.sync.dma_start(
                            out=cf[:, cc, :],
                            in_=cc_out[b][cc % HPC][cc // HPC, :, :],
                        )
                    cf_tiles[b] = cf
                    if b == 0:
                        for tb in range(TOK_SHARD // TOKB):
                            for nb in range(HID // 512):
                                out_filler.append(
                                    lambda tb=tb, nb=nb: emit_out_group(0, tb, nb, cf_tiles[0])
                                )

                # drain remaining b=0 groups, then the b=1 output GEMM
                while out_filler:
                    out_filler.pop(0)()
                for tb in range(TOK_SHARD // TOKB):
                    for nb in range(HID // 512):
                        emit_out_group(1, tb, nb, cf_tiles[1])

    nc.compile()
    return nc


_CACHE = {}


def _get_nc(apply_mask: bool):
    if apply_mask not in _CACHE:
        _CACHE[apply_mask] = _build(apply_mask)
    return _CACHE[apply_mask]


def _prep_in_maps(input, input_mask, norm_w, norm_b, attn_qkvw, attn_qkvb, attn_ow):
    bf16 = ml_dtypes.bfloat16
    x = np.ascontiguousarray(
        np.asarray(input, dtype=np.float32).reshape(T, HID).astype(bf16)
    )
    w = np.asarray(attn_qkvw, dtype=np.float32)
    nw = np.asarray(norm_w, dtype=np.float32)
    nb = np.asarray(norm_b, dtype=np.float32)
    qb_ = np.asarray(attn_qkvb, dtype=np.float32)
    ow = np.ascontiguousarray(np.asarray(attn_ow, dtype=np.float32).astype(bf16))
    mask = np.asarray(input_mask, dtype=np.float32).reshape(B, S)

    w_eff = nw[:, None] * w  # fold LN gamma into QKV weight
    b_eff = nb @ w + qb_  # fold LN beta into QKV bias

    apply_mask = bool(np.any(mask != 0.0))
    in_maps = []
    for i in range(N_CORES):
        cols = []
        for part in range(3):  # q, k, v column shards for this core's heads
            c0 = part * HID + i * HPC * HD
            cols.append(w_eff[:, c0 : c0 + HPC * HD])
        wqkv_i = np.ascontiguousarray(np.concatenate(cols, axis=1).astype(bf16))

        bq = b_eff[i * HPC * HD : (i + 1) * HPC * HD].reshape(HPC, HD)
        bk = b_eff[HID + i * HPC * HD : HID + (i + 1) * HPC * HD].reshape(HPC, HD)
        qkb_i = np.ascontiguousarray(np.stack([bq[0], bq[1], bk[0], bk[1]], axis=1))  # [128, 4]
        vb_i = np.ascontiguousarray(
            b_eff[2 * HID + i * HPC * HD : 2 * HID + (i + 1) * HPC * HD]
            .reshape(1, HPC * HD)
            .astype(bf16)
        )
        m = {"input": x, "qkvw": wqkv_i, "qkb": qkb_i, "vb": vb_i, "ow": ow}
        if apply_mask:
            # [p, b*N_KB + kb] = mask[b, kb*128 + p]
            m["imask"] = np.ascontiguousarray(
                mask.reshape(B, N_KB, 128).transpose(2, 0, 1).reshape(128, B * N_KB)
            )
        in_maps.append(m)
    return in_maps, apply_mask


def _run(inputs: dict, trace: bool = False):
    from concourse.bass_utils import run_bass_kernel_spmd

    in_maps, apply_mask = _prep_in_maps(**inputs)
    nc = _get_nc(apply_mask)
    res = run_bass_kernel_spmd(nc, in_maps, list(range(N_CORES)), trace=trace)
    out = np.empty((B, S, HID), dtype=np.float32)
    for j in range(N_CORES):
        o = res.results[j]["out"]
        for b in range(B):
            out[b, j * TOK_SHARD : (j + 1) * TOK_SHARD] = o[b * TOK_SHARD : (b + 1) * TOK_SHARD]
    return out, res


def kernel(**inputs) -> np.ndarray:
    out, _ = _run(inputs, trace=False)
    return out
